# revision 23
# baseline (speedup 1.0000x reference)
"""MoE layer (top-2 of 8 experts, SwiGLU FFN) on 8 trn2 NeuronCores.

Strategy: expert parallelism. Each core owns one expert. The host computes
only the top-2 *selection* (index lists) and performs the dispatch/combine
data movement (gather tokens per expert / scatter-add partial outputs); all
floating-point math that produces output values — gate logits, top-2
softmax weights, the SwiGLU FFN — runs on device.

Device kernel (identical program on all 8 cores, per-core data):
  inputs   xt    [D, C]  gathered tokens for this expert, transposed
           gw    [D, E]  gate weights, columns rotated so own expert = col 0
           w1,w3 [D, F]  expert FFN in-projections
           w2    [F, D]  expert FFN out-projection
           valid [C]     1.0 for real tokens, 0.0 for padding
  output   yt    [D, C]  weighted expert contribution (transposed)

  per token tile (<=512 tokens):
    logitsT[8, TT] = gw.T @ xT          (PE)
    transpose to [tok, 8], top-2 softmax weight of own expert   (DVE/ACT)
    broadcast weight across partitions via DVE block-transpose + selector
    matmul                                                       (DVE/PE)
    hT[F, TT] = silu(w1.T @ xT) * (w3.T @ xT)                    (PE/ACT/DVE)
    yT[D, TT] = (w2.T)_chunks @ hT, scaled by the gate weight    (PE/DVE)
"""

import numpy as np

T, D, F, E = 8192, 1024, 4096, 8
NCORES = 8
P = 128
TOK_TILE = 512

_nc_cache: dict = {}

# "fp32r": PE multiplies in the hardware's relaxed-fp32 mode (1 cycle/row vs
# 4 for exact fp32), fp32 accumulate in PSUM. "fp32": exact but 4x slower.
MM_MODE = "fp32r"


def _build(C: int, mm_mode: str = MM_MODE):
    """Build + compile the per-core Bass program for capacity C (multiple of 128).

    Token-chunk x F-half blocking: tokens are processed in chunks of up to
    1280 (x and the F-half of hT stay resident in SBUF); for each chunk the
    two F-halves of w1/w3/w2 are streamed exactly once, so total weight
    traffic is one pass per token chunk (~2 passes for C~2304) instead of
    one pass per 512-token tile. The second F-half's output is combined via
    DMA accumulate into the yt DRAM tensor.
    """
    from contextlib import ExitStack

    import concourse.tile as tile
    from concourse import bacc, mybir
    from concourse.bass import ds

    f32 = mybir.dt.float32
    dx = mybir.dt.float32r if mm_mode == "fp32r" else f32
    KD, KF = D // P, F // P
    KH = KF // 2
    X = mybir.AxisListType.X
    Sigmoid = mybir.ActivationFunctionType.Sigmoid
    Exp = mybir.ActivationFunctionType.Exp
    Alu = mybir.AluOpType

    nc = bacc.Bacc(
        "TRN2", target_bir_lowering=False, debug=False, num_devices=NCORES
    )
    xt = nc.dram_tensor("xt", [D, C], dx, kind="ExternalInput")
    gw = nc.dram_tensor("gw", [D, E], dx, kind="ExternalInput")
    w1 = nc.dram_tensor("w1", [D, F], dx, kind="ExternalInput")
    w3 = nc.dram_tensor("w3", [D, F], dx, kind="ExternalInput")
    w2 = nc.dram_tensor("w2", [F, D], dx, kind="ExternalInput")
    vd = nc.dram_tensor("valid", [C], f32, kind="ExternalInput")
    yt = nc.dram_tensor("yt", [D, C], f32, kind="ExternalOutput")

    # chunk plan: token chunks <= 1280, each split into tiles <= 512,
    # sub-512 tile (if any) first within its chunk.
    CHUNK = 1280
    nchunks = -(-C // CHUNK)
    base = (C // nchunks) // P * P
    sizes = [base] * nchunks
    for i in range((C - base * nchunks) // P):
        sizes[i] += P
    chunks = []
    t0 = 0
    for cs in sizes:
        rem = cs % TOK_TILE
        tiles = ([(t0 + cs - rem, rem)] if rem else []) + [
            (t, TOK_TILE) for t in range(t0, t0 + cs - rem, TOK_TILE)
        ]
        chunks.append((t0, cs, tiles))
        t0 += cs

    with ExitStack() as ctx:
        tc = ctx.enter_context(tile.TileContext(nc))
        const = ctx.enter_context(tc.tile_pool(name="const", bufs=1))
        xp = ctx.enter_context(tc.tile_pool(name="xp", bufs=1))
        wp = ctx.enter_context(tc.tile_pool(name="wp", bufs=3))
        hp = ctx.enter_context(tc.tile_pool(name="hp", bufs=1))
        yp = ctx.enter_context(tc.tile_pool(name="yp", bufs=3))
        gp = ctx.enter_context(tc.tile_pool(name="gp", bufs=2))
        psA = ctx.enter_context(tc.tile_pool(name="psA", bufs=2, space="PSUM"))
        psG = ctx.enter_context(tc.tile_pool(name="psG", bufs=1, space="PSUM"))
        psB = ctx.enter_context(tc.tile_pool(name="psB", bufs=2, space="PSUM"))

        # constants
        gw_sb = const.tile([P, KD, E], dx)
        nc.sync.dma_start(gw_sb[:], gw[:, :].rearrange("(ko p) e -> p ko e", p=P))
        valid_sb = const.tile([P, C // P], f32)
        nc.sync.dma_start(valid_sb[:], vd[:].rearrange("(o p) -> p o", p=P))
        # selector row: picks partition 0 of the rhs in the broadcast matmul
        sel_sb = const.tile([32, P], f32)
        nc.vector.memset(sel_sb[:], 0.0)
        nc.vector.memset(sel_sb[0:1, :], 1.0)

        for c0, CS, tiles in chunks:
            x_sb = xp.tile([P, KD, CS], dx, tag="x", name=f"x_{c0}")
            for kd in range(KD):
                nc.sync.dma_start(x_sb[:, kd, :], xt[ds(kd * P, P), ds(c0, CS)])
            wb_all = gp.tile([P, CS], f32, tag="wb_all", name=f"wba_{c0}")

            # ---- gating per tile: top-2 softmax weight of own expert ----
            for t0, TT in tiles:
                S = TT // P
                r0 = t0 - c0
                lt_ps = psG.tile([E, TT], f32, tag="lt", name=f"lt_{t0}")
                for kd in range(KD):
                    nc.tensor.matmul(
                        lt_ps[:],
                        gw_sb[:, kd, :],
                        x_sb[:, kd, ds(r0, TT)],
                        start=(kd == 0),
                        stop=(kd == KD - 1),
                    )
                lt32 = gp.tile([32, TT], f32, tag="lt32", name=f"lt32_{t0}")
                nc.vector.memset(lt32[:], 0.0)
                nc.vector.tensor_copy(lt32[0:E, :], lt_ps[:])
                lg = gp.tile([P, S, 32], f32, tag="lg", name=f"lg_{t0}")
                for s in range(S):
                    for j in range(4):
                        nc.vector.transpose(
                            lg[ds(32 * j, 32), s],
                            lt32[:, ds(s * P + 32 * j, 32)],
                        )
                L = lg[:, :, 0:E]
                m1 = gp.tile([P, S, 1], f32, tag="m1", name=f"m1_{t0}")
                nc.vector.reduce_max(m1[:], L, axis=X)
                dd = gp.tile([P, S, E], f32, tag="d", name=f"d_{t0}")
                nc.vector.tensor_tensor(
                    dd[:], L, m1[:].to_broadcast((P, S, E)), Alu.subtract
                )
                msk = gp.tile([P, S, E], f32, tag="msk", name=f"msk_{t0}")
                nc.vector.tensor_scalar(msk[:], dd[:], 0.0, None, Alu.is_ge)
                nc.vector.tensor_scalar(
                    msk[:], msk[:], -100000.0, None, Alu.mult
                )
                nc.vector.tensor_add(msk[:], msk[:], dd[:])
                m2 = gp.tile([P, S, 1], f32, tag="m2", name=f"m2_{t0}")
                nc.vector.reduce_max(m2[:], msk[:], axis=X)
                e2 = gp.tile([P, S, 1], f32, tag="e2", name=f"e2_{t0}")
                nc.scalar.activation(e2[:], m2[:], Exp)
                den = gp.tile([P, S, 1], f32, tag="den", name=f"den_{t0}")
                nc.vector.tensor_scalar(den[:], e2[:], 1.0, None, Alu.add)
                rec = gp.tile([P, S, 1], f32, tag="rec", name=f"rec_{t0}")
                nc.vector.reciprocal(rec[:], den[:])
                e0 = gp.tile([P, S, 1], f32, tag="e0", name=f"e0_{t0}")
                nc.scalar.activation(e0[:], dd[:, :, 0:1], Exp)
                wgt = gp.tile([P, S, 1], f32, tag="wgt", name=f"wgt_{t0}")
                nc.vector.tensor_mul(wgt[:], e0[:], rec[:])

                wb_ps = psG.tile([P, TT], f32, tag="wb", name=f"wbps_{t0}")
                for s in range(S):
                    wcol = gp.tile(
                        [P, 32], f32, tag="wcol", name=f"wcol_{t0}_{s}"
                    )
                    nc.vector.memset(wcol[:, 1:32], 0.0)
                    nc.vector.tensor_mul(
                        wcol[:, 0:1],
                        wgt[:, s],
                        valid_sb[:, t0 // P + s, None],
                    )
                    wrt = gp.tile([32, P], f32, tag="wrt", name=f"wrt_{t0}_{s}")
                    for j in range(4):
                        nc.vector.transpose(
                            wrt[:, ds(32 * j, 32)], wcol[ds(32 * j, 32), :]
                        )
                    nc.tensor.matmul(
                        wb_ps[:, ds(s * P, P)],
                        sel_sb[:],
                        wrt[:],
                        start=True,
                        stop=True,
                    )
                nc.vector.tensor_copy(wb_all[:, ds(r0, TT)], wb_ps[:])

            for fh in range(2):
                # ---- phase A: hT(F-half) = silu(w1.T x) * (w3.T x) ----
                h_sb = hp.tile([P, KH, CS], dx, tag="h", name=f"h_{c0}_{fh}")
                for fl in range(KH):
                    f = fh * KH + fl
                    w1_sb = wp.tile(
                        [P, KD, P], dx, tag="w1", name=f"w1_{c0}_{f}"
                    )
                    nc.sync.dma_start(
                        w1_sb[:],
                        w1[:, ds(f * P, P)].rearrange(
                            "(ko p) m -> p ko m", p=P
                        ),
                    )
                    w3_sb = wp.tile(
                        [P, KD, P], dx, tag="w3", name=f"w3_{c0}_{f}"
                    )
                    nc.sync.dma_start(
                        w3_sb[:],
                        w3[:, ds(f * P, P)].rearrange(
                            "(ko p) m -> p ko m", p=P
                        ),
                    )
                    for t0, TT in tiles:
                        r0 = t0 - c0
                        h1 = psA.tile(
                            [P, TT], f32, tag="h1", name=f"ph1_{t0}_{f}"
                        )
                        h3 = psA.tile(
                            [P, TT], f32, tag="h3", name=f"ph3_{t0}_{f}"
                        )
                        for kd in range(KD):
                            nc.tensor.matmul(
                                h1[:],
                                w1_sb[:, kd, :],
                                x_sb[:, kd, ds(r0, TT)],
                                start=(kd == 0),
                                stop=(kd == KD - 1),
                            )
                        for kd in range(KD):
                            nc.tensor.matmul(
                                h3[:],
                                w3_sb[:, kd, :],
                                x_sb[:, kd, ds(r0, TT)],
                                start=(kd == 0),
                                stop=(kd == KD - 1),
                            )
                        sg = gp.tile([P, TT], f32, tag="sg", name=f"sg_{t0}_{f}")
                        nc.scalar.activation(sg[:], h1[:], Sigmoid)
                        s1 = gp.tile([P, TT], f32, tag="s1", name=f"s1_{t0}_{f}")
                        nc.vector.tensor_mul(s1[:], sg[:], h1[:])
                        nc.vector.tensor_mul(
                            h_sb[:, fl, ds(r0, TT)], s1[:], h3[:]
                        )

                # ---- phase B: yT(+=) (w2-half.T @ h) * wb ----
                for dm in range(KD):
                    w2_sb = wp.tile(
                        [P, KH, P], dx, tag="w2", name=f"w2_{c0}_{fh}_{dm}"
                    )
                    nc.sync.dma_start(
                        w2_sb[:],
                        w2[ds(fh * KH * P, KH * P), ds(dm * P, P)].rearrange(
                            "(fo p) m -> p fo m", p=P
                        ),
                    )
                    for t0, TT in tiles:
                        r0 = t0 - c0
                        yps = psB.tile(
                            [P, TT], f32, tag="y", name=f"y_{t0}_{fh}_{dm}"
                        )
                        for fk in range(KH):
                            nc.tensor.matmul(
                                yps[:],
                                w2_sb[:, fk, :],
                                h_sb[:, fk, ds(r0, TT)],
                                start=(fk == 0),
                                stop=(fk == KH - 1),
                            )
                        y_sb = yp.tile(
                            [P, TT], f32, tag="y_sb", name=f"ysb_{t0}_{fh}_{dm}"
                        )
                        nc.vector.tensor_mul(
                            y_sb[:], yps[:], wb_all[:, ds(r0, TT)]
                        )
                        if fh == 0:
                            nc.gpsimd.dma_start(
                                yt[ds(dm * P, P), ds(t0, TT)], y_sb[:]
                            )
                        else:
                            nc.gpsimd.dma_start(
                                yt[ds(dm * P, P), ds(t0, TT)],
                                y_sb[:],
                                accum_op=Alu.add,
                            )

    nc.compile()
    return nc


def _route(x: np.ndarray, gw: np.ndarray):
    """Top-2 expert selection (host; indices only — no output values)."""
    logits = x @ gw
    n = x.shape[0]
    top1 = np.argmax(logits, axis=1)
    l2 = logits.copy()
    l2[np.arange(n), top1] = -np.inf
    top2 = np.argmax(l2, axis=1)
    idx = [
        np.nonzero((top1 == e) | (top2 == e))[0].astype(np.int64)
        for e in range(gw.shape[1])
    ]
    return idx


def kernel(x, gate_w, w1, w2, w3, _trace=False, _trace_cores=None, _result_box=None):
    from concourse.bass_utils import run_bass_kernel_spmd

    x = np.ascontiguousarray(np.asarray(x, dtype=np.float32))
    gw = np.ascontiguousarray(np.asarray(gate_w, dtype=np.float32))
    w1 = np.ascontiguousarray(np.asarray(w1, dtype=np.float32))
    w2 = np.ascontiguousarray(np.asarray(w2, dtype=np.float32))
    w3 = np.ascontiguousarray(np.asarray(w3, dtype=np.float32))
    assert x.shape == (T, D) and gw.shape == (D, E), (x.shape, gw.shape)
    assert w1.shape == (E, D, F) and w3.shape == (E, D, F), (w1.shape,)
    assert w2.shape == (E, F, D), (w2.shape,)

    idx = _route(x, gw)
    maxn = max(len(i) for i in idx)
    C = max(P, -(-maxn // P) * P)

    key = (C, MM_MODE)
    if key not in _nc_cache:
        _nc_cache[key] = _build(C)
    nc = _nc_cache[key]

    rot = np.arange(E)
    in_maps = []
    for e in range(E):
        n = len(idx[e])
        xt = np.zeros((D, C), np.float32)
        xt[:, :n] = x[idx[e]].T
        valid = np.zeros((C,), np.float32)
        valid[:n] = 1.0
        in_maps.append(
            {
                "xt": xt,
                "gw": np.ascontiguousarray(gw[:, (rot + e) % E]),
                "w1": w1[e],
                "w3": w3[e],
                "w2": w2[e],
                "valid": valid,
            }
        )

    res = run_bass_kernel_spmd(
        nc,
        in_maps,
        core_ids=list(range(NCORES)),
        trace=_trace,
        trace_cores=_trace_cores,
    )
    if _result_box is not None:
        _result_box.append(res)

    out = np.zeros((T, D), np.float32)
    for e in range(E):
        n = len(idx[e])
        yt = np.asarray(res.results[e]["yt"])
        out[idx[e]] += yt[:, :n].T
    return out


# revision 24
# speedup vs baseline: 1.1946x; 1.1946x over previous
"""MoE layer (top-2 of 8 experts, SwiGLU FFN) on 8 trn2 NeuronCores.

Strategy: expert parallelism. Each core owns one expert. The host computes
only the top-2 *selection* (index lists) and performs the dispatch/combine
data movement (gather tokens per expert / scatter-add partial outputs); all
floating-point math that produces output values — gate logits, top-2
softmax weights, the SwiGLU FFN — runs on device.

Device kernel (identical program on all 8 cores, per-core data):
  inputs   xt    [D, C]  gathered tokens for this expert, transposed
           gw    [D, E]  gate weights, columns rotated so own expert = col 0
           w1,w3 [D, F]  expert FFN in-projections
           w2    [F, D]  expert FFN out-projection
           valid [C]     1.0 for real tokens, 0.0 for padding
  output   yt    [D, C]  weighted expert contribution (transposed)

  per token tile (<=512 tokens):
    logitsT[8, TT] = gw.T @ xT          (PE)
    transpose to [tok, 8], top-2 softmax weight of own expert   (DVE/ACT)
    broadcast weight across partitions via DVE block-transpose + selector
    matmul                                                       (DVE/PE)
    hT[F, TT] = silu(w1.T @ xT) * (w3.T @ xT)                    (PE/ACT/DVE)
    yT[D, TT] = (w2.T)_chunks @ hT, scaled by the gate weight    (PE/DVE)
"""

import numpy as np

T, D, F, E = 8192, 1024, 4096, 8
NCORES = 8
P = 128
TOK_TILE = 512

_nc_cache: dict = {}

# "fp32r": PE multiplies in the hardware's relaxed-fp32 mode (1 cycle/row vs
# 4 for exact fp32), fp32 accumulate in PSUM. "fp32": exact but 4x slower.
MM_MODE = "fp32r"


def _build(C: int, mm_mode: str = MM_MODE):
    """Build + compile the per-core Bass program for capacity C (multiple of 128).

    Token-chunk x F-half blocking: tokens are processed in chunks of up to
    1280 (x and the F-half of hT stay resident in SBUF); for each chunk the
    two F-halves of w1/w3/w2 are streamed exactly once, so total weight
    traffic is one pass per token chunk (~2 passes for C~2304) instead of
    one pass per 512-token tile. The second F-half's output is combined via
    DMA accumulate into the yt DRAM tensor.
    """
    from contextlib import ExitStack

    import concourse.tile as tile
    from concourse import bacc, mybir
    from concourse.bass import ds

    f32 = mybir.dt.float32
    dx = mybir.dt.float32r if mm_mode == "fp32r" else f32
    KD, KF = D // P, F // P
    KH = KF // 2
    X = mybir.AxisListType.X
    Sigmoid = mybir.ActivationFunctionType.Sigmoid
    Exp = mybir.ActivationFunctionType.Exp
    Alu = mybir.AluOpType

    nc = bacc.Bacc(
        "TRN2", target_bir_lowering=False, debug=False, num_devices=NCORES
    )
    xt = nc.dram_tensor("xt", [D, C], dx, kind="ExternalInput")
    gw = nc.dram_tensor("gw", [D, E], dx, kind="ExternalInput")
    w1 = nc.dram_tensor("w1", [D, F], dx, kind="ExternalInput")
    w3 = nc.dram_tensor("w3", [D, F], dx, kind="ExternalInput")
    w2 = nc.dram_tensor("w2", [F, D], dx, kind="ExternalInput")
    vd = nc.dram_tensor("valid", [C], f32, kind="ExternalInput")
    yt = nc.dram_tensor("yt", [D, C], f32, kind="ExternalOutput")

    # chunk plan: token chunks <= 1280, each split into tiles <= 512,
    # sub-512 tile (if any) first within its chunk.
    CHUNK = 1280
    nchunks = -(-C // CHUNK)
    base = (C // nchunks) // P * P
    sizes = [base] * nchunks
    for i in range((C - base * nchunks) // P):
        sizes[i] += P
    chunks = []
    t0 = 0
    for cs in sizes:
        rem = cs % TOK_TILE
        tiles = ([(t0 + cs - rem, rem)] if rem else []) + [
            (t, TOK_TILE) for t in range(t0, t0 + cs - rem, TOK_TILE)
        ]
        chunks.append((t0, cs, tiles))
        t0 += cs

    with ExitStack() as ctx:
        tc = ctx.enter_context(tile.TileContext(nc))
        const = ctx.enter_context(tc.tile_pool(name="const", bufs=1))
        xp = ctx.enter_context(tc.tile_pool(name="xp", bufs=1))
        wp = ctx.enter_context(tc.tile_pool(name="wp", bufs=3))
        hp = ctx.enter_context(tc.tile_pool(name="hp", bufs=1))
        yp = ctx.enter_context(tc.tile_pool(name="yp", bufs=3))
        gp = ctx.enter_context(tc.tile_pool(name="gp", bufs=2))
        psA = ctx.enter_context(tc.tile_pool(name="psA", bufs=2, space="PSUM"))
        psG = ctx.enter_context(tc.tile_pool(name="psG", bufs=1, space="PSUM"))
        psB = ctx.enter_context(tc.tile_pool(name="psB", bufs=2, space="PSUM"))

        # constants
        gw_sb = const.tile([P, KD, E], dx)
        nc.sync.dma_start(gw_sb[:], gw[:, :].rearrange("(ko p) e -> p ko e", p=P))
        valid_sb = const.tile([P, C // P], f32)
        nc.sync.dma_start(valid_sb[:], vd[:].rearrange("(o p) -> p o", p=P))
        # selector row: picks partition 0 of the rhs in the broadcast matmul
        sel_sb = const.tile([32, P], f32)
        nc.vector.memset(sel_sb[:], 0.0)
        nc.vector.memset(sel_sb[0:1, :], 1.0)

        for c0, CS, tiles in chunks:
            x_sb = xp.tile([P, KD, CS], dx, tag="x", name=f"x_{c0}")
            nc.sync.dma_start(
                x_sb[:], xt[:, ds(c0, CS)].rearrange("(ko p) t -> p ko t", p=P)
            )
            wb_all = gp.tile([P, CS], f32, tag="wb_all", name=f"wba_{c0}")

            # ---- gating per tile: top-2 softmax weight of own expert ----
            for t0, TT in tiles:
                S = TT // P
                r0 = t0 - c0
                lt_ps = psG.tile([E, TT], f32, tag="lt", name=f"lt_{t0}")
                for kd in range(KD):
                    nc.tensor.matmul(
                        lt_ps[:],
                        gw_sb[:, kd, :],
                        x_sb[:, kd, ds(r0, TT)],
                        start=(kd == 0),
                        stop=(kd == KD - 1),
                    )
                lt32 = gp.tile([32, TT], f32, tag="lt32", name=f"lt32_{t0}")
                nc.vector.memset(lt32[:], 0.0)
                nc.vector.tensor_copy(lt32[0:E, :], lt_ps[:])
                lg = gp.tile([P, S, 32], f32, tag="lg", name=f"lg_{t0}")
                for s in range(S):
                    for j in range(4):
                        nc.vector.transpose(
                            lg[ds(32 * j, 32), s],
                            lt32[:, ds(s * P + 32 * j, 32)],
                        )
                L = lg[:, :, 0:E]
                m1 = gp.tile([P, S, 1], f32, tag="m1", name=f"m1_{t0}")
                nc.vector.reduce_max(m1[:], L, axis=X)
                dd = gp.tile([P, S, E], f32, tag="d", name=f"d_{t0}")
                nc.vector.tensor_tensor(
                    dd[:], L, m1[:].to_broadcast((P, S, E)), Alu.subtract
                )
                msk = gp.tile([P, S, E], f32, tag="msk", name=f"msk_{t0}")
                nc.vector.tensor_scalar(msk[:], dd[:], 0.0, None, Alu.is_ge)
                nc.vector.tensor_scalar(
                    msk[:], msk[:], -100000.0, None, Alu.mult
                )
                nc.vector.tensor_add(msk[:], msk[:], dd[:])
                m2 = gp.tile([P, S, 1], f32, tag="m2", name=f"m2_{t0}")
                nc.vector.reduce_max(m2[:], msk[:], axis=X)
                e2 = gp.tile([P, S, 1], f32, tag="e2", name=f"e2_{t0}")
                nc.scalar.activation(e2[:], m2[:], Exp)
                den = gp.tile([P, S, 1], f32, tag="den", name=f"den_{t0}")
                nc.vector.tensor_scalar(den[:], e2[:], 1.0, None, Alu.add)
                rec = gp.tile([P, S, 1], f32, tag="rec", name=f"rec_{t0}")
                nc.vector.reciprocal(rec[:], den[:])
                e0 = gp.tile([P, S, 1], f32, tag="e0", name=f"e0_{t0}")
                nc.scalar.activation(e0[:], dd[:, :, 0:1], Exp)
                wgt = gp.tile([P, S, 1], f32, tag="wgt", name=f"wgt_{t0}")
                nc.vector.tensor_mul(wgt[:], e0[:], rec[:])

                wb_ps = psG.tile([P, TT], f32, tag="wb", name=f"wbps_{t0}")
                for s in range(S):
                    wcol = gp.tile(
                        [P, 32], f32, tag="wcol", name=f"wcol_{t0}_{s}"
                    )
                    nc.vector.memset(wcol[:, 1:32], 0.0)
                    nc.vector.tensor_mul(
                        wcol[:, 0:1],
                        wgt[:, s],
                        valid_sb[:, t0 // P + s, None],
                    )
                    wrt = gp.tile([32, P], f32, tag="wrt", name=f"wrt_{t0}_{s}")
                    for j in range(4):
                        nc.vector.transpose(
                            wrt[:, ds(32 * j, 32)], wcol[ds(32 * j, 32), :]
                        )
                    nc.tensor.matmul(
                        wb_ps[:, ds(s * P, P)],
                        sel_sb[:],
                        wrt[:],
                        start=True,
                        stop=True,
                    )
                nc.vector.tensor_copy(wb_all[:, ds(r0, TT)], wb_ps[:])

            for fh in range(2):
                # ---- phase A: hT(F-half) = silu(w1.T x) * (w3.T x) ----
                h_sb = hp.tile([P, KH, CS], dx, tag="h", name=f"h_{c0}_{fh}")
                for fl in range(KH):
                    f = fh * KH + fl
                    w1_sb = wp.tile(
                        [P, KD, P], dx, tag="w1", name=f"w1_{c0}_{f}"
                    )
                    nc.sync.dma_start(
                        w1_sb[:],
                        w1[:, ds(f * P, P)].rearrange(
                            "(ko p) m -> p ko m", p=P
                        ),
                    )
                    w3_sb = wp.tile(
                        [P, KD, P], dx, tag="w3", name=f"w3_{c0}_{f}"
                    )
                    nc.sync.dma_start(
                        w3_sb[:],
                        w3[:, ds(f * P, P)].rearrange(
                            "(ko p) m -> p ko m", p=P
                        ),
                    )
                    for t0, TT in tiles:
                        r0 = t0 - c0
                        h1 = psA.tile(
                            [P, TT], f32, tag="h1", name=f"ph1_{t0}_{f}"
                        )
                        h3 = psA.tile(
                            [P, TT], f32, tag="h3", name=f"ph3_{t0}_{f}"
                        )
                        for kd in range(KD):
                            nc.tensor.matmul(
                                h1[:],
                                w1_sb[:, kd, :],
                                x_sb[:, kd, ds(r0, TT)],
                                start=(kd == 0),
                                stop=(kd == KD - 1),
                            )
                        for kd in range(KD):
                            nc.tensor.matmul(
                                h3[:],
                                w3_sb[:, kd, :],
                                x_sb[:, kd, ds(r0, TT)],
                                start=(kd == 0),
                                stop=(kd == KD - 1),
                            )
                        sg = gp.tile([P, TT], f32, tag="sg", name=f"sg_{t0}_{f}")
                        nc.scalar.activation(sg[:], h1[:], Sigmoid)
                        s1 = gp.tile([P, TT], f32, tag="s1", name=f"s1_{t0}_{f}")
                        nc.vector.tensor_mul(s1[:], sg[:], h1[:])
                        nc.vector.tensor_mul(
                            h_sb[:, fl, ds(r0, TT)], s1[:], h3[:]
                        )

                # ---- phase B: yT(+=) (w2-half.T @ h) * wb ----
                for dm in range(KD):
                    w2_sb = wp.tile(
                        [P, KH, P], dx, tag="w2", name=f"w2_{c0}_{fh}_{dm}"
                    )
                    nc.sync.dma_start(
                        w2_sb[:],
                        w2[ds(fh * KH * P, KH * P), ds(dm * P, P)].rearrange(
                            "(fo p) m -> p fo m", p=P
                        ),
                    )
                    for t0, TT in tiles:
                        r0 = t0 - c0
                        yps = psB.tile(
                            [P, TT], f32, tag="y", name=f"y_{t0}_{fh}_{dm}"
                        )
                        for fk in range(KH):
                            nc.tensor.matmul(
                                yps[:],
                                w2_sb[:, fk, :],
                                h_sb[:, fk, ds(r0, TT)],
                                start=(fk == 0),
                                stop=(fk == KH - 1),
                            )
                        y_sb = yp.tile(
                            [P, TT], f32, tag="y_sb", name=f"ysb_{t0}_{fh}_{dm}"
                        )
                        nc.vector.tensor_mul(
                            y_sb[:], yps[:], wb_all[:, ds(r0, TT)]
                        )
                        if fh == 0:
                            nc.gpsimd.dma_start(
                                yt[ds(dm * P, P), ds(t0, TT)], y_sb[:]
                            )
                        else:
                            nc.gpsimd.dma_start(
                                yt[ds(dm * P, P), ds(t0, TT)],
                                y_sb[:],
                                accum_op=Alu.add,
                            )

    nc.compile()
    return nc


def _route(x: np.ndarray, gw: np.ndarray):
    """Top-2 expert selection (host; indices only — no output values)."""
    logits = x @ gw
    n = x.shape[0]
    top1 = np.argmax(logits, axis=1)
    l2 = logits.copy()
    l2[np.arange(n), top1] = -np.inf
    top2 = np.argmax(l2, axis=1)
    idx = [
        np.nonzero((top1 == e) | (top2 == e))[0].astype(np.int64)
        for e in range(gw.shape[1])
    ]
    return idx


def kernel(x, gate_w, w1, w2, w3, _trace=False, _trace_cores=None, _result_box=None):
    from concourse.bass_utils import run_bass_kernel_spmd

    x = np.ascontiguousarray(np.asarray(x, dtype=np.float32))
    gw = np.ascontiguousarray(np.asarray(gate_w, dtype=np.float32))
    w1 = np.ascontiguousarray(np.asarray(w1, dtype=np.float32))
    w2 = np.ascontiguousarray(np.asarray(w2, dtype=np.float32))
    w3 = np.ascontiguousarray(np.asarray(w3, dtype=np.float32))
    assert x.shape == (T, D) and gw.shape == (D, E), (x.shape, gw.shape)
    assert w1.shape == (E, D, F) and w3.shape == (E, D, F), (w1.shape,)
    assert w2.shape == (E, F, D), (w2.shape,)

    idx = _route(x, gw)
    maxn = max(len(i) for i in idx)
    C = max(P, -(-maxn // P) * P)

    key = (C, MM_MODE)
    if key not in _nc_cache:
        _nc_cache[key] = _build(C)
    nc = _nc_cache[key]

    rot = np.arange(E)
    in_maps = []
    for e in range(E):
        n = len(idx[e])
        xt = np.zeros((D, C), np.float32)
        xt[:, :n] = x[idx[e]].T
        valid = np.zeros((C,), np.float32)
        valid[:n] = 1.0
        in_maps.append(
            {
                "xt": xt,
                "gw": np.ascontiguousarray(gw[:, (rot + e) % E]),
                "w1": w1[e],
                "w3": w3[e],
                "w2": w2[e],
                "valid": valid,
            }
        )

    res = run_bass_kernel_spmd(
        nc,
        in_maps,
        core_ids=list(range(NCORES)),
        trace=_trace,
        trace_cores=_trace_cores,
    )
    if _result_box is not None:
        _result_box.append(res)

    out = np.zeros((T, D), np.float32)
    for e in range(E):
        n = len(idx[e])
        yt = np.asarray(res.results[e]["yt"])
        out[idx[e]] += yt[:, :n].T
    return out
